# revision 1
# baseline (speedup 1.0000x reference)
"""Trainium2 Bass kernel for nn_BlockAttnRes (fused RMSNorm-softmax pooling).

Reference computation (all fp32):
    V = concat([blocks, partial[None]], axis=0)          # (8, B, T, D)
    K = V * rsqrt(mean(V^2, -1) + eps) * norm_weight
    logits  = einsum('d,nbtd->nbt', w, K)
    weights = softmax(logits, axis=0)                    # over the 8 sources
    out     = einsum('nbt,nbtd->btd', weights, V)        # (B, T, D)

Key algebraic reduction: K never needs materializing.
    logits[n,tok] = (sum_d wn[d]*V[n,tok,d]) * rsqrt(mean_d(V^2) + eps)
with wn = w * norm_weight (folded on host). So per (source, token) we need
two free-axis reductions over D, an 8-way softmax, and a weighted sum --
a single pass over V from HBM (memory-roofline).

Sharding: data-parallel over the 8192 tokens (B*T) across 8 NeuronCores,
1024 tokens each, no cross-core communication.

Per-core structure: 4 "supertiles" of 256 tokens; each DMA moves 1 MiB
(one source x 256 tokens) for bandwidth efficiency. Compute runs per
128-token half:
  ACT  : Square+accumulate -> s2;  Exp;  diagonal-weight tiles for PE
  DVE  : part of the dot products (fused scalar_tensor_tensor + accum),
         rsqrt bit-trick + Newton, softmax smalls (batched per supertile),
         final PSUM->SBUF FMA (one fused op per half)
  POOL : the other part of the dot products (gpsimd scalar_tensor_tensor)
  PE   : weighted sum via diag(wgt_n) @ V_n accumulated in PSUM (fp32)
  DMA  : 8x1MiB loads + 1x1MiB store per supertile
"""

import os
import sys

import numpy as np

sys.path.insert(0, "/opt/trn_rl_repo")

N_BLOCKS, B, T, D = 7, 4, 2048, 1024
N_SRC = N_BLOCKS + 1          # 8 sources after appending `partial`
N_CORES = 8
TOK_TOTAL = B * T             # 8192
TOK_PER_CORE = TOK_TOTAL // N_CORES   # 1024
P = 128                       # SBUF partitions
HALVES = 2                    # token-tiles per supertile
ST = P * HALVES               # supertile token count (256)
N_ST = TOK_PER_CORE // ST     # supertiles per core (4)
EPS = float(np.finfo(np.float32).eps)
MAGIC_P1 = 0x5F3759DF + 1     # rsqrt bit-trick constant (+1 for xor-negate form)

# engine split knobs (env-overridable for benchmarking); defaults are the
# hardware-swept optimum: no PE (fp32 matmul is 4 cyc/row — a net loss for
# the weighted sum), 4 dots offloaded to POOL+ACT, 2 squares moved to DVE.
N_PE = int(os.environ.get("KERNEL_N_PE", "0"))       # sources on PE path
N_POOL = int(os.environ.get("KERNEL_N_POOL", "4"))   # dots via POOL-mult + ACT-accum
NR_ITERS = int(os.environ.get("KERNEL_NR", "2"))     # rsqrt Newton steps
DUAL_RING = int(os.environ.get("KERNEL_DUAL_RING", "1"))
N_SQ_DVE = int(os.environ.get("KERNEL_N_SQ_DVE", "2"))  # squares on DVE instead of ACT
SPLIT_CHAIN = int(os.environ.get("KERNEL_SPLIT_CHAIN", "0"))  # 2 FMA sub-chains

_STATE: dict = {}


def _split_multi_waits(nc):
    """TPB instructions encode a single sem-wait; this walrus build refuses
    instructions carrying more (`Too many sync wait commands`). Split extra
    waits onto single-wait NoOps on the same engine, preserving per-engine
    program order (and therefore semantics)."""
    import concourse.mybir as mybir

    for fn in nc.m.functions:
        for blk in fn.blocks:
            insts = list(blk.instructions)
            out = []
            changed = False
            for ins in insts:
                si = ins.sync_info
                if si is not None and len(si.on_wait) > 1:
                    waits = list(si.on_wait)
                    for k, w in enumerate(waits[:-1]):
                        nop = mybir.InstNoOp(name=f"{ins.name}-sw{k}", ins=[], outs=[])
                        nop.engine = ins.engine
                        nop.sync_info = mybir.SyncInfo(on_wait=[w], on_update=[])
                        out.append(nop)
                    ins.sync_info = mybir.SyncInfo(
                        on_wait=[waits[-1]], on_update=list(si.on_update)
                    )
                    changed = True
                out.append(ins)
            if changed:
                blk.instructions = out
    return nc


def _build_nc(
    n_pe: int = N_PE,
    repeat: int = 1,
    loop: bool = True,
    mode: str = "full",
    n_pool: int = N_POOL,
    n_sq_dve: int = N_SQ_DVE,
):
    import concourse.bass as bass
    import concourse.mybir as mybir
    import concourse.tile as tile
    from contextlib import ExitStack

    f32 = mybir.dt.float32
    i32 = mybir.dt.int32
    Alu = mybir.AluOpType
    Act = mybir.ActivationFunctionType

    nc = bass.Bass("TRN2", target_bir_lowering=False, debug=False)

    blocks_d = nc.dram_tensor(
        "blocks", [N_BLOCKS, TOK_PER_CORE, D], f32, kind="ExternalInput"
    )
    partial_d = nc.dram_tensor("partial", [TOK_PER_CORE, D], f32, kind="ExternalInput")
    wn_d = nc.dram_tensor("wn", [P, D], f32, kind="ExternalInput")
    ident_d = nc.dram_tensor("ident", [P, P], f32, kind="ExternalInput")
    out_d = nc.dram_tensor("out", [TOK_PER_CORE, D], f32, kind="ExternalOutput")

    # (h p) d -> p h d views so one DMA moves a full 1MiB supertile slice
    # into a [128, 2, 1024] SBUF tile.
    bap = blocks_d.ap().rearrange("n (s h p) d -> n s p h d", p=P, h=HALVES)
    pap = partial_d.ap().rearrange("(s h p) d -> s p h d", p=P, h=HALVES)
    oap = out_d.ap().rearrange("(s h p) d -> s p h d", p=P, h=HALVES)

    pe_src = list(range(N_SRC - n_pe, N_SRC))   # sources on the PE path
    dve_src = list(range(0, N_SRC - n_pe))      # weighted-sum leftovers on DVE
    pool_dots = set(range(N_SRC - n_pool, N_SRC))  # dot products on GPSIMD

    NST2 = N_SRC * HALVES  # stats tile width: 8 sources x 2 halves

    with tile.TileContext(nc) as tc, ExitStack() as ctx:
        const_pool = ctx.enter_context(tc.tile_pool(name="const", bufs=1))
        vpool = ctx.enter_context(tc.tile_pool(name="v", bufs=2))
        scr_pool = ctx.enter_context(tc.tile_pool(name="scr", bufs=2))
        stat_pool = ctx.enter_context(tc.tile_pool(name="stat", bufs=3))
        diag_pool = ctx.enter_context(tc.tile_pool(name="diag", bufs=2))
        out_pool = ctx.enter_context(tc.tile_pool(name="outp", bufs=2))
        psum_pool = ctx.enter_context(tc.tile_pool(name="ps", bufs=2, space="PSUM"))

        wn_sb = const_pool.tile([P, D], f32, name="wn_sb")
        nc.sync.dma_start(wn_sb[:], wn_d.ap()[:, :])
        if n_pe:
            ident_sb = const_pool.tile([P, P], f32, name="ident_sb")
            nc.sync.dma_start(ident_sb[:], ident_d.ap()[:, :])

        def ring(k):
            # alternate HWDGE rings: SP (qSPDynamicHW) and ACT (qActDynamicHW)
            if DUAL_RING and (k % 2 == 1):
                return nc.scalar
            return nc.sync

        def supertile_body(s, r=0):
            v = []
            for n in range(N_BLOCKS):
                vt = vpool.tile([P, HALVES, D], f32, tag=f"v{n}", name=f"v{n}_{s}_{r}")
                ring(n).dma_start(vt[:], bap[n, s])
                v.append(vt)
            vt = vpool.tile([P, HALVES, D], f32, tag="v7", name=f"v7_{s}_{r}")
            ring(7).dma_start(vt[:], pap[s])
            v.append(vt)

            acc = out_pool.tile([P, HALVES, D], f32, tag="acc", name=f"acc_{s}_{r}")

            if mode == "dma":
                ring(8).dma_start(oap[s], v[7][:])
                return

            # ---- stats: s2 (ACT Square+accum) and dot products ----
            # dot split: `pool_dots` sources go POOL-multiply + ACT-Copy-accum
            # (keeps DVE free); the rest are single fused DVE STT+accum ops.
            # Full-tensor outputs that nothing reads are written to 0-stride
            # broadcast dummies (saves SBUF; the accumulator output is what
            # matters).
            s2 = stat_pool.tile([P, NST2], f32, tag="s2", name=f"s2_{s}")
            dot = stat_pool.tile([P, NST2], f32, tag="dot", name=f"dot_{s}")
            dum_act = scr_pool.tile([P, 1], f32, tag="dumA", name=f"dumA_{s}")
            dum_dve = scr_pool.tile([P, 1], f32, tag="dumV", name=f"dumV_{s}")
            for h in range(HALVES):
                for n in range(N_SRC):
                    col = h * N_SRC + n
                    if n < n_sq_dve:
                        nc.vector.scalar_tensor_tensor(
                            out=dum_dve[:].broadcast_to((P, D)),
                            in0=v[n][:, h, :],
                            scalar=1.0,
                            in1=v[n][:, h, :],
                            op0=Alu.mult,
                            op1=Alu.mult,
                            accum_out=s2[:, col : col + 1],
                        )
                    else:
                        nc.scalar.activation(
                            dum_act[:].broadcast_to((P, D)),
                            v[n][:, h, :],
                            Act.Square,
                            accum_out=s2[:, col : col + 1],
                        )
                    if n in pool_dots:
                        pr_scr = scr_pool.tile(
                            [P, D], f32, tag="prg", name=f"prg_{s}_{col}", bufs=4
                        )
                        nc.gpsimd.tensor_tensor(
                            pr_scr[:], v[n][:, h, :], wn_sb[:], Alu.mult
                        )
                        nc.scalar.activation(
                            dum_act[:].broadcast_to((P, D)),
                            pr_scr[:],
                            Act.Copy,
                            accum_out=dot[:, col : col + 1],
                        )
                    else:
                        nc.vector.scalar_tensor_tensor(
                            out=dum_dve[:].broadcast_to((P, D)),
                            in0=v[n][:, h, :],
                            scalar=1.0,
                            in1=wn_sb[:],
                            op0=Alu.mult,
                            op1=Alu.mult,
                            accum_out=dot[:, col : col + 1],
                        )

            # ---- softmax over the 8 sources, batched over both halves ----
            # ms = s2/D + eps ; y = rsqrt(ms) via bit trick + Newton
            ms = stat_pool.tile([P, NST2], f32, tag="ms", name=f"ms_{s}")
            nc.vector.tensor_scalar(ms[:], s2[:], 1.0 / D, EPS, Alu.mult, Alu.add)
            ti = stat_pool.tile([P, NST2], i32, tag="ti", name=f"ti_{s}")
            nc.vector.tensor_single_scalar(
                ti[:], ms[:].bitcast(i32), 1, Alu.logical_shift_right
            )
            y = stat_pool.tile([P, NST2], f32, tag="y", name=f"y_{s}")
            nc.vector.tensor_single_scalar(ti[:], ti[:], -1, Alu.bitwise_xor)
            nc.vector.tensor_single_scalar(y[:].bitcast(i32), ti[:], MAGIC_P1, Alu.add)
            for it in range(NR_ITERS):
                a = stat_pool.tile([P, NST2], f32, tag="nra", name=f"nra_{s}_{it}")
                nc.vector.tensor_tensor(a[:], y[:], y[:], Alu.mult)
                nc.vector.tensor_tensor(a[:], a[:], ms[:], Alu.mult)
                nc.vector.tensor_scalar(a[:], a[:], -0.5, 1.5, Alu.mult, Alu.add)
                nc.vector.tensor_tensor(y[:], y[:], a[:], Alu.mult)

            lg = stat_pool.tile([P, HALVES, N_SRC], f32, tag="lg", name=f"lg_{s}")
            nc.vector.tensor_tensor(
                lg[:].rearrange("p h n -> p (h n)"), dot[:], y[:], Alu.mult
            )

            nm = stat_pool.tile([P, HALVES, 1], f32, tag="nm", name=f"nm_{s}")
            nc.vector.tensor_reduce(
                nm[:], lg[:], axis=mybir.AxisListType.X, op=Alu.max, negate=True
            )
            lgs = stat_pool.tile([P, HALVES, N_SRC], f32, tag="lgs", name=f"lgs_{s}")
            nc.vector.tensor_tensor(
                lgs[:], lg[:], nm[:].broadcast_to((P, HALVES, N_SRC)), Alu.add
            )
            e = stat_pool.tile([P, HALVES, N_SRC], f32, tag="e", name=f"e_{s}")
            nc.scalar.activation(
                e[:].rearrange("p h n -> p (h n)"),
                lgs[:].rearrange("p h n -> p (h n)"),
                Act.Exp,
            )
            den = stat_pool.tile([P, HALVES, 1], f32, tag="den", name=f"den_{s}")
            nc.vector.tensor_reduce(den[:], e[:], axis=mybir.AxisListType.X, op=Alu.add)
            rcp = stat_pool.tile([P, HALVES, 1], f32, tag="rcp", name=f"rcp_{s}")
            nc.vector.reciprocal(rcp[:], den[:])
            wgt = stat_pool.tile([P, HALVES, N_SRC], f32, tag="wgt", name=f"wgt_{s}")
            nc.vector.tensor_tensor(
                wgt[:], e[:], rcp[:].broadcast_to((P, HALVES, N_SRC)), Alu.mult
            )

            # ---- weighted sum ----
            for h in range(HALVES):
                chain = list(dve_src)
                ps = None
                if pe_src:
                    diags = []
                    for j, n in enumerate(pe_src):
                        dg = diag_pool.tile(
                            [P, P], f32, tag=f"dg{j}", name=f"dg{j}_{s}_{h}"
                        )
                        nc.scalar.activation(
                            dg[:], ident_sb[:], Act.Copy, scale=wgt[:, h, n : n + 1]
                        )
                        diags.append(dg)
                    ps = psum_pool.tile([P, D], f32, tag="ps", name=f"ps_{s}_{h}")
                    half = D // 2
                    for c in range(2):
                        cs = slice(c * half, (c + 1) * half)
                        for j, n in enumerate(pe_src):
                            nc.tensor.matmul(
                                ps[:, cs],
                                lhsT=diags[j][:],
                                rhs=v[n][:, h, cs],
                                start=(j == 0),
                                stop=(j == len(pe_src) - 1),
                            )
                # the remaining sources fold in via fused DVE FMA; when a PE
                # partial exists, the first FMA also evacuates PSUM -> SBUF
                if ps is not None:
                    if not chain:
                        nc.vector.tensor_single_scalar(
                            acc[:, h, :], ps[:], 1.0, Alu.mult
                        )
                        continue
                    n0 = chain.pop(0)
                    nc.vector.scalar_tensor_tensor(
                        out=acc[:, h, :],
                        in0=v[n0][:, h, :],
                        scalar=wgt[:, h, n0 : n0 + 1],
                        in1=ps[:],
                        op0=Alu.mult,
                        op1=Alu.add,
                    )
                else:
                    n0 = chain.pop(0)
                    nc.vector.tensor_single_scalar(
                        acc[:, h, :], v[n0][:, h, :], wgt[:, h, n0 : n0 + 1], Alu.mult
                    )
                if SPLIT_CHAIN and len(chain) >= 4:
                    # two independent FMA sub-chains halve dependency depth
                    mid = len(chain) // 2
                    acc2 = out_pool.tile(
                        [P, D], f32, tag="acc2", name=f"acc2_{s}_{h}_{r}"
                    )
                    nB = chain[mid]
                    nc.vector.tensor_single_scalar(
                        acc2[:], v[nB][:, h, :], wgt[:, h, nB : nB + 1], Alu.mult
                    )
                    for n in chain[mid + 1 :]:
                        nc.vector.scalar_tensor_tensor(
                            out=acc2[:],
                            in0=v[n][:, h, :],
                            scalar=wgt[:, h, n : n + 1],
                            in1=acc2[:],
                            op0=Alu.mult,
                            op1=Alu.add,
                        )
                    chain = chain[:mid]
                else:
                    acc2 = None
                for n in chain:
                    nc.vector.scalar_tensor_tensor(
                        out=acc[:, h, :],
                        in0=v[n][:, h, :],
                        scalar=wgt[:, h, n : n + 1],
                        in1=acc[:, h, :],
                        op0=Alu.mult,
                        op1=Alu.add,
                    )
                if acc2 is not None:
                    nc.vector.tensor_tensor(
                        acc[:, h, :], acc[:, h, :], acc2[:], Alu.add
                    )

            ring(8).dma_start(oap[s], acc[:])

        if repeat == 1:
            for s in range(N_ST):
                supertile_body(s)
        elif loop:
            # benchmark mode: re-run the whole per-core computation `repeat`
            # times inside a hardware loop so marginal wall-clock isolates
            # on-device execution time from axon/PJRT dispatch overhead.
            with tc.For_i(0, repeat, 1):
                for s in range(N_ST):
                    supertile_body(s)
        else:
            for r in range(repeat):
                for s in range(N_ST):
                    supertile_body(s, r)

    return _split_multi_waits(nc)


def _get_state():
    if "nc" not in _STATE:
        _STATE["nc"] = _build_nc()
    return _STATE["nc"]


def _prepare_in_maps(blocks, partial, norm_weight, w):
    blocks = np.asarray(blocks, dtype=np.float32)
    partial = np.asarray(partial, dtype=np.float32)
    norm_weight = np.asarray(norm_weight, dtype=np.float32)
    w = np.asarray(w, dtype=np.float32)

    wn = (w * norm_weight).astype(np.float32)
    wn_b = np.ascontiguousarray(np.broadcast_to(wn, (P, D)))
    ident = np.eye(P, dtype=np.float32)

    blocks_f = blocks.reshape(N_BLOCKS, TOK_TOTAL, D)
    partial_f = partial.reshape(TOK_TOTAL, D)

    in_maps = []
    for c in range(N_CORES):
        sl = slice(c * TOK_PER_CORE, (c + 1) * TOK_PER_CORE)
        in_maps.append(
            {
                "blocks": np.ascontiguousarray(blocks_f[:, sl, :]),
                "partial": np.ascontiguousarray(partial_f[sl, :]),
                "wn": wn_b,
                "ident": ident,
            }
        )
    return in_maps


def _run(inputs, trace=False, **kwargs):
    from concourse.bass_utils import run_bass_kernel_spmd

    nc = _get_state()
    in_maps = _prepare_in_maps(**inputs)
    bkr = run_bass_kernel_spmd(
        nc, in_maps, core_ids=list(range(N_CORES)), trace=trace, **kwargs
    )
    out = np.concatenate([bkr.results[c]["out"] for c in range(N_CORES)], axis=0)
    return out.reshape(B, T, D), bkr


def kernel(**inputs) -> np.ndarray:
    out, _ = _run(inputs, trace=False)
    return out



# revision 3
# speedup vs baseline: 1.7033x; 1.7033x over previous
"""Trainium2 Bass kernel for nn_BlockAttnRes — v4 (fp32, 128-token tiles,
PE fp32r weighted sum, deep buffering).

Reference computation (all fp32):
    V = concat([blocks, partial[None]], axis=0)          # (8, B, T, D)
    K = V * rsqrt(mean(V^2, -1) + eps) * norm_weight
    logits  = einsum('d,nbtd->nbt', w, K)
    weights = softmax(logits, axis=0)                    # over the 8 sources
    out     = einsum('nbt,nbtd->btd', weights, V)        # (B, T, D)

v4 structure (per core: 8 token-tiles of 128 tokens):
  - fp32 HWDGE loads, 0.5 MiB per (source, tile), alternating sync/scalar
    rings (SWDGE breaks walrus codegen inside tc.For_i; fp32r removes any
    need for cast-loads). 4-deep v buffering decouples load/compute/PE.
  - stats: squares on ACT (Square+accum), dots on DVE (fused STT+accum).
  - softmax: rsqrt bit-trick + Newton on DVE (sqrt(D) folded into wn, eps
    dropped); max-subtraction folded into ACT Exp bias; 1/den deferred to
    the PSUM-evac scale operand (den/rcp off the critical path).
  - weighted sum on PE in fp32r (1 cyc/row @ N=512): diag(e_n) @ V_n in
    PSUM; diag blocks built in one DVE mult from an identity-blocks const.
  - PSUM evac+normalize via ACT Copy(scale=1/den); fp32 HWDGE store.
"""

import os
import sys

import numpy as np

sys.path.insert(0, "/opt/trn_rl_repo")

N_BLOCKS, B, T, D = 7, 4, 2048, 1024
N_SRC = N_BLOCKS + 1          # 8 sources after appending `partial`
N_CORES = 8
TOK_TOTAL = B * T             # 8192
TOK_PER_CORE = TOK_TOTAL // N_CORES   # 1024
P = 128                       # SBUF partitions
N_TT = TOK_PER_CORE // P      # token-tiles per core (8)
MAGIC_P1 = 0x5F3759DF + 1     # rsqrt bit-trick constant (+1 for xor-negate form)

N_SQ_DVE = int(os.environ.get("K4_N_SQ_DVE", "0"))   # sources whose squares run on DVE
NR_ITERS = int(os.environ.get("K4_NR", "2"))         # rsqrt Newton steps
V_BUFS = int(os.environ.get("K4_V_BUFS", "4"))       # v-tile buffering
PS_BUFS = int(os.environ.get("K4_PS_BUFS", "3"))     # PSUM buffering
DIAG_POOL = int(os.environ.get("K4_DIAG_POOL", "1")) # diag build on GPSIMD

_STATE: dict = {}


def _split_multi_waits(nc):
    """TPB instructions encode a single sem-wait; split extra waits onto
    single-wait NoOps on the same engine, preserving per-engine order."""
    import concourse.mybir as mybir

    for fn in nc.m.functions:
        for blk in fn.blocks:
            insts = list(blk.instructions)
            out = []
            changed = False
            for ins in insts:
                si = ins.sync_info
                if si is not None and len(si.on_wait) > 1:
                    waits = list(si.on_wait)
                    for k, w in enumerate(waits[:-1]):
                        nop = mybir.InstNoOp(name=f"{ins.name}-sw{k}", ins=[], outs=[])
                        nop.engine = ins.engine
                        nop.sync_info = mybir.SyncInfo(on_wait=[w], on_update=[])
                        out.append(nop)
                    ins.sync_info = mybir.SyncInfo(
                        on_wait=[waits[-1]], on_update=list(si.on_update)
                    )
                    changed = True
                out.append(ins)
            if changed:
                blk.instructions = out
    return nc


def _build_nc(
    repeat: int = 1,
    loop: bool = True,
    unroll: int = 4,
    mode: str = "full",
    n_sq_dve: int = N_SQ_DVE,
    v_bufs: int = V_BUFS,
    ps_bufs: int = PS_BUFS,
    diag_pool_eng: int = DIAG_POOL,
):
    import concourse.bass as bass
    import concourse.mybir as mybir
    import concourse.tile as tile
    from contextlib import ExitStack

    f32 = mybir.dt.float32
    f32r = mybir.dt.float32r
    i32 = mybir.dt.int32
    Alu = mybir.AluOpType
    Act = mybir.ActivationFunctionType

    nc = bass.Bass("TRN2", target_bir_lowering=False, debug=False)

    blocks_d = nc.dram_tensor(
        "blocks", [N_BLOCKS, TOK_PER_CORE, D], f32, kind="ExternalInput"
    )
    partial_d = nc.dram_tensor("partial", [TOK_PER_CORE, D], f32, kind="ExternalInput")
    wn_d = nc.dram_tensor("wn", [P, D], f32, kind="ExternalInput")
    ident8_d = nc.dram_tensor("ident8", [P, N_SRC * P], f32, kind="ExternalInput")
    out_d = nc.dram_tensor("out", [TOK_PER_CORE, D], f32, kind="ExternalOutput")

    bap = blocks_d.ap().rearrange("n (t p) d -> n t p d", p=P)
    pap = partial_d.ap().rearrange("(t p) d -> t p d", p=P)
    oap = out_d.ap().rearrange("(t p) d -> t p d", p=P)

    with tile.TileContext(nc) as tc, ExitStack() as ctx:
        const_pool = ctx.enter_context(tc.tile_pool(name="const", bufs=1))
        vpool = ctx.enter_context(tc.tile_pool(name="v", bufs=v_bufs))
        scr_pool = ctx.enter_context(tc.tile_pool(name="scr", bufs=2))
        stat_pool = ctx.enter_context(tc.tile_pool(name="stat", bufs=4))
        diag_pool = ctx.enter_context(tc.tile_pool(name="diag", bufs=3))
        out_pool = ctx.enter_context(tc.tile_pool(name="outp", bufs=3))
        psum_pool = ctx.enter_context(
            tc.tile_pool(name="ps", bufs=ps_bufs, space="PSUM")
        )

        wn_sb = const_pool.tile([P, D], f32, name="wn_sb")
        nc.sync.dma_start(wn_sb[:], wn_d.ap()[:, :])
        ident8_sb = const_pool.tile([P, N_SRC, P], f32, name="ident8_sb")
        nc.sync.dma_start(
            ident8_sb[:].rearrange("p n q -> p (n q)"), ident8_d.ap()[:, :]
        )

        def ring(k):
            # all loads on the SP (sync) HWDGE ring, all stores on the ACT
            # (scalar) ring: a store waits on the whole pipeline, and a
            # not-ready store at the head of a ring blocks every later load
            # behind it (FIFO per ring).
            return nc.sync

        def store_ring(k):
            return nc.scalar

        def stage_front(t, r=0):
            """loads + stats + softmax + diag build for token-tile t."""
            v = []
            for n in range(N_BLOCKS):
                vt = vpool.tile([P, D], f32, tag=f"v{n}", name=f"v{n}_{t}_{r}")
                ring(n).dma_start(vt[:].bitcast(f32r), bap[n, t].bitcast(f32r))
                v.append(vt)
            vt = vpool.tile([P, D], f32, tag="v7", name=f"v7_{t}_{r}")
            ring(7).dma_start(vt[:].bitcast(f32r), pap[t].bitcast(f32r))
            v.append(vt)

            if mode == "dma":
                return v

            # ---- stats: s2 (squares) and dots, accumulated over D ----
            s2 = stat_pool.tile([P, N_SRC], f32, tag="s2", name=f"s2_{t}")
            dot = stat_pool.tile([P, N_SRC], f32, tag="dot", name=f"dot_{t}")
            dum_act = scr_pool.tile([P, 1], f32, tag="dumA", name=f"dumA_{t}")
            dum_dve = scr_pool.tile([P, 1], f32, tag="dumV", name=f"dumV_{t}")
            for n in range(N_SRC):
                if n < n_sq_dve:
                    nc.vector.scalar_tensor_tensor(
                        out=dum_dve[:].broadcast_to((P, D)),
                        in0=v[n][:],
                        scalar=1.0,
                        in1=v[n][:],
                        op0=Alu.mult,
                        op1=Alu.mult,
                        accum_out=s2[:, n : n + 1],
                    )
                else:
                    nc.scalar.activation(
                        dum_act[:].broadcast_to((P, D)),
                        v[n][:],
                        Act.Square,
                        accum_out=s2[:, n : n + 1],
                    )
                nc.vector.scalar_tensor_tensor(
                    out=dum_dve[:].broadcast_to((P, D)),
                    in0=v[n][:],
                    scalar=1.0,
                    in1=wn_sb[:],
                    op0=Alu.mult,
                    op1=Alu.mult,
                    accum_out=dot[:, n : n + 1],
                )

            # ---- softmax over the 8 sources ----
            with tc.high_priority():
                # y = rsqrt(s2): bit trick (2 fused ops) + Newton (3 ops/iter)
                ti = stat_pool.tile([P, N_SRC], i32, tag="ti", name=f"ti_{t}")
                nc.vector.tensor_scalar(
                    ti[:], s2[:].bitcast(i32), 1, -1,
                    Alu.logical_shift_right, Alu.bitwise_xor,
                )
                y = stat_pool.tile([P, N_SRC], f32, tag="y", name=f"y_{t}")
                nc.vector.tensor_single_scalar(
                    y[:].bitcast(i32), ti[:], MAGIC_P1, Alu.add
                )
                for it in range(NR_ITERS):
                    # y' = y * (1.5 + (y*y * -0.5) * s2)
                    a = stat_pool.tile([P, N_SRC], f32, tag="nra", name=f"nra_{t}_{it}")
                    nc.vector.tensor_tensor(a[:], y[:], y[:], Alu.mult)
                    nc.vector.scalar_tensor_tensor(
                        out=a[:], in0=a[:], scalar=-0.5, in1=s2[:],
                        op0=Alu.mult, op1=Alu.mult,
                    )
                    nc.vector.scalar_tensor_tensor(
                        out=y[:], in0=a[:], scalar=1.5, in1=y[:],
                        op0=Alu.add, op1=Alu.mult,
                    )

                lg = stat_pool.tile([P, N_SRC], f32, tag="lg", name=f"lg_{t}")
                nc.vector.tensor_tensor(lg[:], dot[:], y[:], Alu.mult)
                nm = stat_pool.tile([P, 1], f32, tag="nm", name=f"nm_{t}")
                nc.vector.tensor_reduce(
                    nm[:], lg[:], axis=mybir.AxisListType.X, op=Alu.max, negate=True
                )
                # e = exp(lg - max); max-subtraction via ACT's free affine.
                # Normalization deferred to the PSUM-evac scale operand.
                e = stat_pool.tile([P, N_SRC, 1], f32, tag="e", name=f"e_{t}")
                nc.scalar.activation(
                    e[:].rearrange("p n o -> p (n o)"), lg[:], Act.Exp, bias=nm[:]
                )

                # all 8 diag blocks in one op
                dg = diag_pool.tile([P, N_SRC, P], f32r, tag="dg", name=f"dg_{t}")
                diag_eng = nc.gpsimd if diag_pool_eng else nc.vector
                diag_eng.tensor_tensor(
                    dg[:],
                    ident8_sb[:],
                    e[:].broadcast_to((P, N_SRC, P)),
                    Alu.mult,
                )
            # off critical path: den = sum(e), rcp = 1/den (used at evac)
            den = stat_pool.tile([P, 1], f32, tag="den", name=f"den_{t}")
            nc.vector.tensor_reduce(
                den[:],
                e[:].rearrange("p n o -> p (n o)"),
                axis=mybir.AxisListType.X,
                op=Alu.add,
            )
            rcp = stat_pool.tile([P, 1], f32, tag="rcp", name=f"rcp_{t}")
            nc.vector.reciprocal(rcp[:], den[:])
            return v, dg, rcp

        def stage_back(t, state, r=0):
            """weighted sum on PE (fp32r), normalize+evac on ACT, store."""
            if mode == "dma":
                v = state
                store_ring(t).dma_start(oap[t], v[0][:])
                return
            v, dg, rcp = state
            acc = out_pool.tile([P, D], f32, tag="acc", name=f"acc_{t}_{r}")
            half = D // 2
            ps = psum_pool.tile([P, D], f32, tag="ps", name=f"ps_{t}_{r}")
            for c in range(2):
                cs = slice(c * half, (c + 1) * half)
                for j in range(N_SRC):
                    nc.tensor.matmul(
                        ps[:, cs],
                        lhsT=dg[:, j, :],
                        rhs=v[j][:, cs].bitcast(f32r),
                        start=(j == 0),
                        stop=(j == N_SRC - 1),
                    )
            # evac normalizes: acc = ps * (1/den)  (ACT free affine).
            # High priority: PSUM recycling gates the next tiles' matmuls.
            with tc.high_priority():
                nc.scalar.activation(acc[:], ps[:], Act.Copy, scale=rcp[:])
            store_ring(t).dma_start(oap[t], acc[:])

        def pipelined(r=0):
            states = {}
            states[0] = stage_front(0, r)
            for t in range(1, N_TT):
                states[t] = stage_front(t, r)
                stage_back(t - 1, states.pop(t - 1), r)
            stage_back(N_TT - 1, states.pop(N_TT - 1), r)

        if repeat == 1:
            pipelined()
        elif loop:
            # unrolled bodies inside the hardware loop: For_i carries an
            # all-engine barrier per trip, so amortize it over `unroll`
            # full passes (buffer tags cycle across bodies, keeping the
            # pipeline running within a trip).
            assert repeat % unroll == 0, (repeat, unroll)
            with tc.For_i(0, repeat // unroll, 1):
                for u in range(unroll):
                    pipelined(u)
        else:
            for r in range(repeat):
                pipelined(r)

    return _split_multi_waits(nc)


def _get_state():
    if "nc" not in _STATE:
        _STATE["nc"] = _build_nc()
    return _STATE["nc"]


def _prepare_in_maps(blocks, partial, norm_weight, w):
    blocks = np.asarray(blocks, dtype=np.float32)
    partial = np.asarray(partial, dtype=np.float32)
    norm_weight = np.asarray(norm_weight, dtype=np.float32)
    w = np.asarray(w, dtype=np.float32)

    # sqrt(D) folded in: logits = dot(wn*sqrt(D), V) * rsqrt(sum V^2)
    #                           = dot(wn, V) * rsqrt(mean V^2)   (eps dropped)
    wn = (w * norm_weight * np.sqrt(np.float32(D))).astype(np.float32)
    wn_b = np.ascontiguousarray(np.broadcast_to(wn, (P, D)))
    ident8 = np.zeros((P, N_SRC, P), dtype=np.float32)
    for p in range(P):
        ident8[p, :, p] = 1.0
    ident8 = ident8.reshape(P, N_SRC * P)

    blocks_f = blocks.reshape(N_BLOCKS, TOK_TOTAL, D)
    partial_f = partial.reshape(TOK_TOTAL, D)

    in_maps = []
    for c in range(N_CORES):
        sl = slice(c * TOK_PER_CORE, (c + 1) * TOK_PER_CORE)
        in_maps.append(
            {
                "blocks": np.ascontiguousarray(blocks_f[:, sl, :]),
                "partial": np.ascontiguousarray(partial_f[sl, :]),
                "wn": wn_b,
                "ident8": ident8,
            }
        )
    return in_maps


def _run(inputs, trace=False, **kwargs):
    from concourse.bass_utils import run_bass_kernel_spmd

    nc = _get_state()
    in_maps = _prepare_in_maps(**inputs)
    bkr = run_bass_kernel_spmd(
        nc, in_maps, core_ids=list(range(N_CORES)), trace=trace, **kwargs
    )
    out = np.concatenate([bkr.results[c]["out"] for c in range(N_CORES)], axis=0)
    return out.reshape(B, T, D), bkr


def kernel(**inputs) -> np.ndarray:
    out, _ = _run(inputs, trace=False)
    return out


# revision 4
# speedup vs baseline: 1.8409x; 1.0808x over previous
"""Trainium2 Bass kernel for nn_BlockAttnRes — v4 (fp32, 128-token tiles,
PE fp32r weighted sum, deep buffering).

Reference computation (all fp32):
    V = concat([blocks, partial[None]], axis=0)          # (8, B, T, D)
    K = V * rsqrt(mean(V^2, -1) + eps) * norm_weight
    logits  = einsum('d,nbtd->nbt', w, K)
    weights = softmax(logits, axis=0)                    # over the 8 sources
    out     = einsum('nbt,nbtd->btd', weights, V)        # (B, T, D)

v4 structure (per core: 8 token-tiles of 128 tokens):
  - fp32 HWDGE loads, 0.5 MiB per (source, tile), all on the sync ring
    (stores go on the scalar ring so a pipeline-tail store can never
    head-of-line-block later loads). 4-deep v buffering decouples
    load/compute/PE. (SWDGE is avoided: it breaks walrus codegen inside
    tc.For_i, and fp32r removes any need for cast-during-DMA loads.)
  - stats: squares on ACT (Square+accum), dots on DVE (fused STT+accum).
  - softmax: rsqrt bit-trick + Newton on DVE (sqrt(D) folded into wn, eps
    dropped); max-subtraction folded into ACT Exp bias; 1/den deferred to
    the PSUM-evac scale operand (den/rcp off the critical path).
  - weighted sum on PE in fp32r (1 cyc/row @ N=512): diag(e_n) @ V_n in
    PSUM; diag blocks built in one DVE mult from an identity-blocks const.
  - PSUM evac+normalize via ACT Copy(scale=1/den); fp32 HWDGE store.
"""

import os
import sys

import numpy as np

sys.path.insert(0, "/opt/trn_rl_repo")

N_BLOCKS, B, T, D = 7, 4, 2048, 1024
N_SRC = N_BLOCKS + 1          # 8 sources after appending `partial`
N_CORES = 8
TOK_TOTAL = B * T             # 8192
TOK_PER_CORE = TOK_TOTAL // N_CORES   # 1024
P = 128                       # SBUF partitions
N_TT = TOK_PER_CORE // P      # token-tiles per core (8)
MAGIC_P1 = 0x5F3759DF + 1     # rsqrt bit-trick constant (+1 for xor-negate form)

N_SQ_DVE = int(os.environ.get("K4_N_SQ_DVE", "0"))   # sources whose squares run on DVE
NR_ITERS = int(os.environ.get("K4_NR", "2"))         # rsqrt Newton steps
V_BUFS = int(os.environ.get("K4_V_BUFS", "4"))       # v-tile buffering
PS_BUFS = int(os.environ.get("K4_PS_BUFS", "3"))     # PSUM buffering
DIAG_POOL = int(os.environ.get("K4_DIAG_POOL", "1")) # diag build on GPSIMD

_STATE: dict = {}


def _split_multi_waits(nc):
    """TPB instructions encode a single sem-wait; split extra waits onto
    single-wait NoOps on the same engine, preserving per-engine order."""
    import concourse.mybir as mybir

    for fn in nc.m.functions:
        for blk in fn.blocks:
            insts = list(blk.instructions)
            out = []
            changed = False
            for ins in insts:
                si = ins.sync_info
                if si is not None and len(si.on_wait) > 1:
                    waits = list(si.on_wait)
                    for k, w in enumerate(waits[:-1]):
                        nop = mybir.InstNoOp(name=f"{ins.name}-sw{k}", ins=[], outs=[])
                        nop.engine = ins.engine
                        nop.sync_info = mybir.SyncInfo(on_wait=[w], on_update=[])
                        out.append(nop)
                    ins.sync_info = mybir.SyncInfo(
                        on_wait=[waits[-1]], on_update=list(si.on_update)
                    )
                    changed = True
                out.append(ins)
            if changed:
                blk.instructions = out
    return nc


def _build_nc(
    repeat: int = 1,
    loop: bool = True,
    unroll: int = 4,
    mode: str = "full",
    n_sq_dve: int = N_SQ_DVE,
    v_bufs: int = V_BUFS,
    ps_bufs: int = PS_BUFS,
    diag_pool_eng: int = DIAG_POOL,
):
    import concourse.bass as bass
    import concourse.mybir as mybir
    import concourse.tile as tile
    from contextlib import ExitStack

    f32 = mybir.dt.float32
    f32r = mybir.dt.float32r
    i32 = mybir.dt.int32
    Alu = mybir.AluOpType
    Act = mybir.ActivationFunctionType

    nc = bass.Bass("TRN2", target_bir_lowering=False, debug=False)

    blocks_d = nc.dram_tensor(
        "blocks", [N_BLOCKS, TOK_PER_CORE, D], f32, kind="ExternalInput"
    )
    partial_d = nc.dram_tensor("partial", [TOK_PER_CORE, D], f32, kind="ExternalInput")
    wn_d = nc.dram_tensor("wn", [P, D], f32, kind="ExternalInput")
    ident8_d = nc.dram_tensor("ident8", [P, N_SRC * P], f32, kind="ExternalInput")
    out_d = nc.dram_tensor("out", [TOK_PER_CORE, D], f32, kind="ExternalOutput")

    bap = blocks_d.ap().rearrange("n (t p) d -> n t p d", p=P)
    pap = partial_d.ap().rearrange("(t p) d -> t p d", p=P)
    oap = out_d.ap().rearrange("(t p) d -> t p d", p=P)

    with tile.TileContext(nc) as tc, ExitStack() as ctx:
        const_pool = ctx.enter_context(tc.tile_pool(name="const", bufs=1))
        vpool = ctx.enter_context(tc.tile_pool(name="v", bufs=v_bufs))
        scr_pool = ctx.enter_context(tc.tile_pool(name="scr", bufs=2))
        stat_pool = ctx.enter_context(tc.tile_pool(name="stat", bufs=4))
        diag_pool = ctx.enter_context(tc.tile_pool(name="diag", bufs=3))
        out_pool = ctx.enter_context(tc.tile_pool(name="outp", bufs=3))
        psum_pool = ctx.enter_context(
            tc.tile_pool(name="ps", bufs=ps_bufs, space="PSUM")
        )

        wn_sb = const_pool.tile([P, D], f32, name="wn_sb")
        nc.sync.dma_start(wn_sb[:], wn_d.ap()[:, :])
        ident8_sb = const_pool.tile([P, N_SRC, P], f32, name="ident8_sb")
        nc.sync.dma_start(
            ident8_sb[:].rearrange("p n q -> p (n q)"), ident8_d.ap()[:, :]
        )

        def ring(k):
            # all loads on the SP (sync) HWDGE ring, all stores on the ACT
            # (scalar) ring: a store waits on the whole pipeline, and a
            # not-ready store at the head of a ring blocks every later load
            # behind it (FIFO per ring).
            return nc.sync

        def store_ring(k):
            return nc.scalar

        def stage_front(t, r=0):
            """loads + stats + softmax + diag build for token-tile t."""
            v = []
            for n in range(N_BLOCKS):
                vt = vpool.tile([P, D], f32, tag=f"v{n}", name=f"v{n}_{t}_{r}")
                ring(n).dma_start(vt[:].bitcast(f32r), bap[n, t].bitcast(f32r))
                v.append(vt)
            vt = vpool.tile([P, D], f32, tag="v7", name=f"v7_{t}_{r}")
            ring(7).dma_start(vt[:].bitcast(f32r), pap[t].bitcast(f32r))
            v.append(vt)

            if mode == "dma":
                return v

            # ---- stats: s2 (squares) and dots, accumulated over D ----
            s2 = stat_pool.tile([P, N_SRC], f32, tag="s2", name=f"s2_{t}")
            dot = stat_pool.tile([P, N_SRC], f32, tag="dot", name=f"dot_{t}")
            dum_act = scr_pool.tile([P, 1], f32, tag="dumA", name=f"dumA_{t}")
            dum_dve = scr_pool.tile([P, 1], f32, tag="dumV", name=f"dumV_{t}")
            for n in range(N_SRC):
                if n < n_sq_dve:
                    nc.vector.scalar_tensor_tensor(
                        out=dum_dve[:].broadcast_to((P, D)),
                        in0=v[n][:],
                        scalar=1.0,
                        in1=v[n][:],
                        op0=Alu.mult,
                        op1=Alu.mult,
                        accum_out=s2[:, n : n + 1],
                    )
                else:
                    nc.scalar.activation(
                        dum_act[:].broadcast_to((P, D)),
                        v[n][:],
                        Act.Square,
                        accum_out=s2[:, n : n + 1],
                    )
                nc.vector.scalar_tensor_tensor(
                    out=dum_dve[:].broadcast_to((P, D)),
                    in0=v[n][:],
                    scalar=1.0,
                    in1=wn_sb[:],
                    op0=Alu.mult,
                    op1=Alu.mult,
                    accum_out=dot[:, n : n + 1],
                )

            # ---- softmax over the 8 sources ----
            with tc.high_priority():
                # y = rsqrt(s2): bit trick (2 fused ops) + Newton (3 ops/iter)
                ti = stat_pool.tile([P, N_SRC], i32, tag="ti", name=f"ti_{t}")
                nc.vector.tensor_scalar(
                    ti[:], s2[:].bitcast(i32), 1, -1,
                    Alu.logical_shift_right, Alu.bitwise_xor,
                )
                y = stat_pool.tile([P, N_SRC], f32, tag="y", name=f"y_{t}")
                nc.vector.tensor_single_scalar(
                    y[:].bitcast(i32), ti[:], MAGIC_P1, Alu.add
                )
                for it in range(NR_ITERS):
                    # y' = y * (1.5 + (y*y * -0.5) * s2)
                    a = stat_pool.tile([P, N_SRC], f32, tag="nra", name=f"nra_{t}_{it}")
                    nc.vector.tensor_tensor(a[:], y[:], y[:], Alu.mult)
                    nc.vector.scalar_tensor_tensor(
                        out=a[:], in0=a[:], scalar=-0.5, in1=s2[:],
                        op0=Alu.mult, op1=Alu.mult,
                    )
                    nc.vector.scalar_tensor_tensor(
                        out=y[:], in0=a[:], scalar=1.5, in1=y[:],
                        op0=Alu.add, op1=Alu.mult,
                    )

                lg = stat_pool.tile([P, N_SRC], f32, tag="lg", name=f"lg_{t}")
                nc.vector.tensor_tensor(lg[:], dot[:], y[:], Alu.mult)
                nm = stat_pool.tile([P, 1], f32, tag="nm", name=f"nm_{t}")
                nc.vector.tensor_reduce(
                    nm[:], lg[:], axis=mybir.AxisListType.X, op=Alu.max, negate=True
                )
                # e = exp(lg - max); max-subtraction via ACT's free affine.
                # Normalization deferred to the PSUM-evac scale operand.
                e = stat_pool.tile([P, N_SRC, 1], f32, tag="e", name=f"e_{t}")
                nc.scalar.activation(
                    e[:].rearrange("p n o -> p (n o)"), lg[:], Act.Exp, bias=nm[:]
                )

                # all 8 diag blocks in one op
                dg = diag_pool.tile([P, N_SRC, P], f32r, tag="dg", name=f"dg_{t}")
                diag_eng = nc.gpsimd if diag_pool_eng else nc.vector
                diag_eng.tensor_tensor(
                    dg[:],
                    ident8_sb[:],
                    e[:].broadcast_to((P, N_SRC, P)),
                    Alu.mult,
                )
            # off critical path: den = sum(e), rcp = 1/den (used at evac)
            den = stat_pool.tile([P, 1], f32, tag="den", name=f"den_{t}")
            nc.vector.tensor_reduce(
                den[:],
                e[:].rearrange("p n o -> p (n o)"),
                axis=mybir.AxisListType.X,
                op=Alu.add,
            )
            rcp = stat_pool.tile([P, 1], f32, tag="rcp", name=f"rcp_{t}")
            nc.vector.reciprocal(rcp[:], den[:])
            return v, dg, rcp

        def stage_back(t, state, r=0):
            """weighted sum on PE (fp32r), normalize+evac on ACT, store."""
            if mode == "dma":
                v = state
                store_ring(t).dma_start(oap[t], v[0][:])
                return
            v, dg, rcp = state
            acc = out_pool.tile([P, D], f32, tag="acc", name=f"acc_{t}_{r}")
            half = D // 2
            ps = psum_pool.tile([P, D], f32, tag="ps", name=f"ps_{t}_{r}")
            for c in range(2):
                cs = slice(c * half, (c + 1) * half)
                for j in range(N_SRC):
                    nc.tensor.matmul(
                        ps[:, cs],
                        lhsT=dg[:, j, :],
                        rhs=v[j][:, cs].bitcast(f32r),
                        start=(j == 0),
                        stop=(j == N_SRC - 1),
                    )
            # evac normalizes: acc = ps * (1/den)  (ACT free affine).
            # High priority: PSUM recycling gates the next tiles' matmuls.
            with tc.high_priority():
                nc.scalar.activation(acc[:], ps[:], Act.Copy, scale=rcp[:])
            store_ring(t).dma_start(oap[t], acc[:])

        def pipelined(r=0):
            states = {}
            states[0] = stage_front(0, r)
            for t in range(1, N_TT):
                states[t] = stage_front(t, r)
                stage_back(t - 1, states.pop(t - 1), r)
            stage_back(N_TT - 1, states.pop(N_TT - 1), r)

        if repeat == 1:
            pipelined()
        elif loop:
            # unrolled bodies inside the hardware loop: For_i carries an
            # all-engine barrier per trip, so amortize it over `unroll`
            # full passes (buffer tags cycle across bodies, keeping the
            # pipeline running within a trip).
            assert repeat % unroll == 0, (repeat, unroll)
            with tc.For_i(0, repeat // unroll, 1):
                for u in range(unroll):
                    pipelined(u)
        else:
            for r in range(repeat):
                pipelined(r)

    return _split_multi_waits(nc)


def _get_state():
    if "nc" not in _STATE:
        _STATE["nc"] = _build_nc()
    return _STATE["nc"]


def _prepare_in_maps(blocks, partial, norm_weight, w):
    blocks = np.asarray(blocks, dtype=np.float32)
    partial = np.asarray(partial, dtype=np.float32)
    norm_weight = np.asarray(norm_weight, dtype=np.float32)
    w = np.asarray(w, dtype=np.float32)

    # sqrt(D) folded in: logits = dot(wn*sqrt(D), V) * rsqrt(sum V^2)
    #                           = dot(wn, V) * rsqrt(mean V^2)   (eps dropped)
    wn = (w * norm_weight * np.sqrt(np.float32(D))).astype(np.float32)
    wn_b = np.ascontiguousarray(np.broadcast_to(wn, (P, D)))
    ident8 = np.zeros((P, N_SRC, P), dtype=np.float32)
    for p in range(P):
        ident8[p, :, p] = 1.0
    ident8 = ident8.reshape(P, N_SRC * P)

    blocks_f = blocks.reshape(N_BLOCKS, TOK_TOTAL, D)
    partial_f = partial.reshape(TOK_TOTAL, D)

    in_maps = []
    for c in range(N_CORES):
        sl = slice(c * TOK_PER_CORE, (c + 1) * TOK_PER_CORE)
        in_maps.append(
            {
                "blocks": np.ascontiguousarray(blocks_f[:, sl, :]),
                "partial": np.ascontiguousarray(partial_f[sl, :]),
                "wn": wn_b,
                "ident8": ident8,
            }
        )
    return in_maps


def _run(inputs, trace=False, **kwargs):
    from concourse.bass_utils import run_bass_kernel_spmd

    nc = _get_state()
    in_maps = _prepare_in_maps(**inputs)
    bkr = run_bass_kernel_spmd(
        nc, in_maps, core_ids=list(range(N_CORES)), trace=trace, **kwargs
    )
    out = np.concatenate([bkr.results[c]["out"] for c in range(N_CORES)], axis=0)
    return out.reshape(B, T, D), bkr


def kernel(**inputs) -> np.ndarray:
    out, _ = _run(inputs, trace=False)
    return out
